# revision 5
# baseline (speedup 1.0000x reference)
"""TRN2 Bass kernel for nn_MultiHeadHyperedgeAttention.

Pipeline (8 NeuronCores, hyperedge-sharded, no collectives):
  host: sort edges by hyperedge; first-fit-decreasing bin packing of
        segments into bins of <=64 slots with <=128 edges per node-shard
        (4 shards of 25000 rows so gather indices fit int16); build
        per-bin wrapped gather-index tables and compact slot/weight
        tables; x converted to fp8(e4m3), rows padded to 256B stride.
  dev:  per chunk (16 bins; tail chunk may be 8):
        - dma_gather fp8 rows (128B descriptors, stride 256B) on 4
          SWDGE queues, one per node-shard
        - build the scaled one-hot M on-device (DVE is_equal/mult with
          3D broadcast APs against an iota constant)
        - per-bin matmuls G^T @ M accumulated over shards in PSUM
          (fp8 x fp8 -> f32), PSUM -> f16 featsT via ACT copies
        - MLP for the previous chunk's slots issued before this chunk's
          segment matmuls (3 matmuls + ACT relu/sigmoid + DVE clip),
          f16 weights, f32 PSUM
  host: scatter slot outputs back to the [50000] output.
"""
import numpy as np

import concourse.bass as bass
import concourse.tile as tile
from concourse import ap_utils, bacc, mybir
from concourse.library_config import mlp as mlp_lib
from concourse.bass_utils import run_bass_kernel_spmd

NUM_NODES = 100000
NUM_HYPEREDGES = 50000
IN_DIM = 128
NUM_HEADS = 8
N_CORES = 8
N_SHARDS = 4
SHARD = NUM_NODES // N_SHARDS      # 25000 rows -> int16-safe gather indices
SLOTS = 64                         # segment slots per bin
BINCAP = 128                       # per-shard edge capacity per bin
KBMAX = 16                         # max bins per chunk
PAD_SLOT = 999.0
P = 128
D = IN_DIM
F32 = mybir.dt.float32
F16 = mybir.dt.float16
F8 = mybir.dt.float8e4
I16 = mybir.dt.int16
XPAD = 256                         # fp8 row stride in bytes (= 256B units)
NP_F8 = mybir.dt.np(F8)
AF = mybir.ActivationFunctionType
OP = mybir.AluOpType
SIG_LO = 1.0 / (1.0 + np.exp(5.0))
SIG_HI = 1.0 / (1.0 + np.exp(-5.0))


# ---------------------------------------------------------------- host packing

def _pack(node_idx, hyperedge_idx, binmul=8):
    node_idx = np.asarray(node_idx, dtype=np.int64)
    hyperedge_idx = np.asarray(hyperedge_idx, dtype=np.int64)
    counts = np.bincount(hyperedge_idx, minlength=NUM_HYPEREDGES)
    inv_cnt = 1.0 / np.maximum(counts, 1).astype(np.float64)

    shard_of_edge = node_idx // SHARD
    order = np.lexsort((node_idx, shard_of_edge, hyperedge_idx))
    e_node = node_idx[order]
    e_shard = shard_of_edge[order]

    cnt_ss = np.zeros((NUM_HYPEREDGES, N_SHARDS), dtype=np.int64)
    np.add.at(cnt_ss, (hyperedge_idx, shard_of_edge), 1)
    seg_starts = np.zeros(NUM_HYPEREDGES + 1, dtype=np.int64)
    seg_starts[1:] = np.cumsum(counts)

    # segments whose per-shard edge count exceeds one bin go to the host
    # fallback path (never happens for the target distribution)
    fallback = np.where(cnt_ss.max(axis=1) > BINCAP)[0]
    fb = set(fallback.tolist())

    seg_per_core = NUM_HYPEREDGES // N_CORES
    cores = []
    for c in range(N_CORES):
        s0, s1 = c * seg_per_core, (c + 1) * seg_per_core
        segs_c = [s for s in range(s0, s1) if s not in fb]
        # first-fit-decreasing vector bin packing: capacity BINCAP per
        # shard, <= SLOTS segments per bin (~97% fill)
        order_c = sorted(segs_c, key=lambda s: -int(cnt_ss[s].max()))
        bin_cnt = np.zeros((0, N_SHARDS), dtype=np.int64)
        bin_segs = []
        bin_free = np.zeros(0, dtype=np.int64)
        for s in order_c:
            csm = cnt_ss[s]
            fits = np.where((bin_free > 0) &
                            ((bin_cnt + csm) <= BINCAP).all(axis=1))[0]
            if len(fits):
                i = int(fits[0])
                bin_cnt[i] += csm
                bin_segs[i].append(s)
                bin_free[i] -= 1
            else:
                bin_cnt = np.concatenate([bin_cnt, csm[None]], axis=0)
                bin_segs.append([s])
                bin_free = np.concatenate([bin_free, [SLOTS - 1]])
        cores.append([(bin_segs[i], bin_cnt[i]) for i in range(len(bin_segs))])

    nbins = max(len(b) for b in cores)
    nbins = -(-nbins // binmul) * binmul

    idx16 = np.zeros((N_CORES, N_SHARDS, nbins, BINCAP), dtype=np.int16)
    slotf = np.full((N_CORES, nbins, BINCAP, N_SHARDS), PAD_SLOT, dtype=np.float32)
    wf = np.zeros((N_CORES, nbins, BINCAP, N_SHARDS), dtype=np.float32)
    out_map = np.full((N_CORES, nbins, SLOTS), -1, dtype=np.int64)

    for c in range(N_CORES):
        for b, (segs, _cnt) in enumerate(cores[c]):
            out_map[c, b, :len(segs)] = segs
            pos = np.zeros(N_SHARDS, dtype=np.int64)
            for sl, s in enumerate(segs):
                e0, e1 = seg_starts[s], seg_starts[s + 1]
                nodes = e_node[e0:e1]
                shards = e_shard[e0:e1]
                for sh in range(N_SHARDS):
                    msk = shards == sh
                    k = int(msk.sum())
                    if k == 0:
                        continue
                    p0 = pos[sh]
                    idx16[c, sh, b, p0:p0 + k] = (nodes[msk] - sh * SHARD).astype(np.int16)
                    slotf[c, b, p0:p0 + k, sh] = sl
                    wf[c, b, p0:p0 + k, sh] = inv_cnt[s]
                    pos[sh] += k
            # sort each shard's 128 positions by node id for HBM locality
            for sh in range(N_SHARDS):
                o = np.argsort(idx16[c, sh, b], kind="stable")
                idx16[c, sh, b] = idx16[c, sh, b][o]
                slotf[c, b, :, sh] = slotf[c, b, o, sh]
                wf[c, b, :, sh] = wf[c, b, o, sh]

    # per-bin wrapped idx blocks (idx i -> partition i%16, col i//16 within
    # any bin-aligned window): [P, N_SHARDS, nbins*8]
    IW = BINCAP // 16
    gidx = np.zeros((N_CORES, P, N_SHARDS, nbins * IW), dtype=np.int16)
    for c in range(N_CORES):
        for sh in range(N_SHARDS):
            w = idx16[c, sh].reshape(nbins, IW, 16).transpose(0, 2, 1)
            blk = w.transpose(1, 0, 2).reshape(16, nbins * IW)
            gidx[c, :, sh, :] = np.tile(blk, (8, 1))

    # compact slot/weight tables: [P, 2, nbins*N_SHARDS], col = b*N_SHARDS+s
    slotw = np.zeros((N_CORES, P, 2, nbins * N_SHARDS), dtype=np.float16)
    slotw[:, :, 0, :] = slotf.transpose(0, 2, 1, 3).reshape(N_CORES, P, -1)
    slotw[:, :, 1, :] = wf.transpose(0, 2, 1, 3).reshape(N_CORES, P, -1)

    meta = dict(nbins=nbins, nslots=nbins * SLOTS)
    return dict(gidx=gidx, slotw=slotw, out_map=out_map,
                fallback=fallback, meta=meta)


def _make_mlp_consts(W1, b1, W2, b2):
    W1 = np.asarray(W1, np.float32); b1 = np.asarray(b1, np.float32)
    W2 = np.asarray(W2, np.float32); b2 = np.asarray(b2, np.float32)
    H, Din, K = W1.shape
    w1cat = np.ascontiguousarray(W1.transpose(1, 0, 2).reshape(Din, H * K))
    w2blk = np.zeros((H * K, H), np.float32)
    for h in range(H):
        w2blk[h * K:(h + 1) * K, h] = W2[h]
    iota64 = np.broadcast_to(np.arange(SLOTS, dtype=np.float16), (P, SLOTS)).copy()
    return dict(w1cat=w1cat.astype(np.float16), b1cat=b1.reshape(H * K, 1),
                w2blk=w2blk.astype(np.float16), b2col=b2.reshape(H, 1),
                meanw=np.full((H, 1), 0.9 / H, np.float16),
                iota64=iota64)


def _make_in_map(core, x, packed, consts):
    im = {
        "gidx": packed["gidx"][core],
        "slotw": packed["slotw"][core],
        **consts,
    }
    for s in range(N_SHARDS):
        xp = np.zeros((SHARD, XPAD), dtype=NP_F8)
        xp[:, :D] = x[s * SHARD:(s + 1) * SHARD].astype(NP_F8)
        im[f"xs{s}"] = xp
    return im


# ---------------------------------------------------------------- device kernel

def _dma_gather_raw(g, out_ap, in_ap, idxs_ap, num_idxs, elem_size, elem_step,
                    queue_num):
    """dma_gather minus the vestigial elem_size_bytes%256 assert
    (non-transpose, HBM source). The descriptor stride field still
    requires elem_step bytes to be a multiple of 256."""
    g._assert_queue_num(queue_num)
    assert idxs_ap.dtype == mybir.dt.int16
    assert in_ap.dtype == out_ap.dtype
    assert in_ap.space == bass.MemorySpace.DRAM
    assert idxs_ap.space == bass.MemorySpace.SBUF
    assert out_ap.space == bass.MemorySpace.SBUF
    assert ap_utils.ap_is_contiguous(out_ap.ap[1:])
    assert ap_utils.ap_is_contiguous(idxs_ap.ap[1:])
    assert in_ap.ap[-1][1] == out_ap.ap[-1][1] == elem_size
    assert out_ap.ap[0][1] * out_ap.ap[1][1] == -(-num_idxs // 128) * 128
    assert in_ap.ap[0][0] == elem_step
    stride_bytes = elem_step * mybir.dt.size(in_ap.dtype)
    assert stride_bytes % 256 == 0
    stride_bytes_256 = stride_bytes // 256
    assert stride_bytes_256 < 256
    _in_ap = g.lower_ap_dma(in_ap, for_custom_bir_dma=True)
    _idxs_ap = g.lower_ap(idxs_ap)
    _out_ap = g.lower_ap(out_ap)
    return g.add_instruction(
        mybir.InstDMAGatherAnt(
            name=g.bass.get_next_instruction_name(),
            ins=[*_in_ap, _idxs_ap, g.lower_val_access(g.to_reg(num_idxs))],
            outs=[_out_ap],
            transpose=False,
            num_idxs=num_idxs,
            elem_size=elem_size,
            stride_bytes_256=stride_bytes_256,
            gen_mode=0,
            single_packet=False,
            queue_num=queue_num,
            sbuf_tokens_per_rank=0,
            sbuf_free_dim_per_rank=0,
            sbuf_free_dim_pad_per_rank=0,
            sbuf_byte_offset=0,
        )
    )


def build_nc(nbins, n_cores, mlp_chunk=512, repeat=1, gbufs=16):
    nslots = nbins * SLOTS
    assert nslots % mlp_chunk == 0
    IW = BINCAP // 16
    chunk_plan = [KBMAX] * (nbins // KBMAX)
    if nbins % KBMAX:
        chunk_plan.append(nbins % KBMAX)
    nc = bacc.Bacc("TRN2", target_bir_lowering=False, debug=False,
                   num_devices=n_cores, num_swdge_queues=4)
    xs = [nc.dram_tensor(f"xs{s}", [SHARD, XPAD], F8, kind="ExternalInput").ap()
          for s in range(N_SHARDS)]
    gidx = nc.dram_tensor("gidx", [P, N_SHARDS, nbins * IW], I16,
                          kind="ExternalInput").ap()
    slotw_d = nc.dram_tensor("slotw", [P, 2, nbins * N_SHARDS], F16,
                             kind="ExternalInput").ap()
    w1_d = nc.dram_tensor("w1cat", [D, 64], F16, kind="ExternalInput").ap()
    b1_d = nc.dram_tensor("b1cat", [64, 1], F32, kind="ExternalInput").ap()
    w2_d = nc.dram_tensor("w2blk", [64, 8], F16, kind="ExternalInput").ap()
    b2_d = nc.dram_tensor("b2col", [8, 1], F32, kind="ExternalInput").ap()
    mean_d = nc.dram_tensor("meanw", [8, 1], F16, kind="ExternalInput").ap()
    iota_d = nc.dram_tensor("iota64", [P, SLOTS], F16, kind="ExternalInput").ap()
    out_d = nc.dram_tensor("out", [1, nslots], F32, kind="ExternalOutput").ap()

    with tile.TileContext(nc) as tc:
        with (
            tc.tile_pool(name="consts", bufs=1) as cpool,
            tc.tile_pool(name="idx", bufs=5) as ipool,
            tc.tile_pool(name="sw", bufs=5) as spool,
            tc.tile_pool(name="g", bufs=gbufs) as gpool,
            tc.tile_pool(name="m4w", bufs=3) as mpool,
            tc.tile_pool(name="eq", bufs=3) as epool,
            tc.tile_pool(name="feats", bufs=1) as fpool,
            tc.tile_pool(name="mlptmp", bufs=3) as tpool,
            tc.tile_pool(name="outp", bufs=1) as opool,
            tc.tile_pool(name="psf", bufs=3, space="PSUM") as psf,
            tc.tile_pool(name="psh", bufs=2, space="PSUM") as psh,
            tc.tile_pool(name="psa", bufs=1, space="PSUM") as psa,
            tc.tile_pool(name="pso", bufs=1, space="PSUM") as pso,
        ):
            nc.gpsimd.load_library(mlp_lib)
            w1_t = cpool.tile([D, 64], F16)
            nc.sync.dma_start(out=w1_t[:], in_=w1_d[:])
            b1_t = cpool.tile([64, 1], F32)
            nc.sync.dma_start(out=b1_t[:], in_=b1_d[:])
            w2_t = cpool.tile([64, 8], F16)
            nc.sync.dma_start(out=w2_t[:], in_=w2_d[:])
            b2_t = cpool.tile([8, 1], F32)
            nc.sync.dma_start(out=b2_t[:], in_=b2_d[:])
            mean_t = cpool.tile([8, 1], F16)
            nc.sync.dma_start(out=mean_t[:], in_=mean_d[:])
            iota_t = cpool.tile([P, SLOTS], F16)
            nc.sync.dma_start(out=iota_t[:], in_=iota_d[:])

            featsT = fpool.tile([P, nslots], F16)
            out_sb = opool.tile([1, nslots], F32)

            def emit_mlp(lo, hi):
                for j in range(lo, hi):
                    cols = slice(j * mlp_chunk, (j + 1) * mlp_chunk)
                    ph = psh.tile([64, mlp_chunk], F32, tag="ph")
                    nc.tensor.matmul(out=ph[:], lhsT=w1_t[:],
                                     rhs=featsT[:, cols], start=True, stop=True)
                    hr = tpool.tile([64, mlp_chunk], F16, tag="hr")
                    nc.scalar.activation(out=hr[:], in_=ph[:], func=AF.Relu,
                                         bias=b1_t[:])
                    pa = psa.tile([8, mlp_chunk], F32, tag="pa")
                    nc.tensor.matmul(out=pa[:], lhsT=w2_t[:], rhs=hr[:],
                                     start=True, stop=True)
                    sg = tpool.tile([8, mlp_chunk], F16, tag="sg")
                    nc.scalar.activation(out=sg[:], in_=pa[:], func=AF.Sigmoid,
                                         bias=b2_t[:])
                    nc.vector.tensor_scalar(out=sg[:], in0=sg[:],
                                            scalar1=float(SIG_LO),
                                            scalar2=float(SIG_HI),
                                            op0=OP.max, op1=OP.min)
                    po = pso.tile([1, mlp_chunk], F32, tag="po")
                    nc.tensor.matmul(out=po[:], lhsT=mean_t[:], rhs=sg[:],
                                     start=True, stop=True)
                    nc.scalar.activation(out=out_sb[:, cols], in_=po[:],
                                         func=AF.Copy, bias=0.1)

            for _r in range(repeat):
                bin0 = 0
                mlp_done = 0
                for ci, kb_ch in enumerate(chunk_plan):
                    # MLP for completed slots (previous chunks) goes first so
                    # PE isn't head-of-line blocked behind gather-dependent
                    # segment matmuls
                    avail = (bin0 * SLOTS) // mlp_chunk
                    emit_mlp(mlp_done, avail)
                    mlp_done = avail
                    nid = kb_ch * BINCAP
                    it = ipool.tile([P, N_SHARDS, KBMAX * IW], I16, tag="idx")
                    nc.sync.dma_start(
                        out=it[:, :, :kb_ch * IW],
                        in_=gidx[:, :, bin0 * IW:(bin0 + kb_ch) * IW])
                    sw = spool.tile([P, 2, KBMAX * N_SHARDS], F16, tag="sw")
                    nc.sync.dma_start(
                        out=sw[:, :, :kb_ch * N_SHARDS],
                        in_=slotw_d[:, :, bin0 * N_SHARDS:(bin0 + kb_ch) * N_SHARDS])
                    gts = []
                    for s in range(N_SHARDS):
                        G = gpool.tile([P, KBMAX, D], F8, tag="G")
                        _dma_gather_raw(
                            nc.gpsimd, G[:, :kb_ch, :], xs[s][:, 0:D],
                            it[:, s, :kb_ch * IW],
                            nid, D, XPAD, queue_num=(s + ci) % 4)
                        gts.append(G)
                    # build the scaled one-hot on-device:
                    #   m4[p, j, t] = (slot[p, j] == t) * w[p, j]
                    NBc = kb_ch * N_SHARDS
                    eq = epool.tile([P, KBMAX * N_SHARDS, SLOTS], F16, tag="eq")
                    slot_b = sw[:, 0, :NBc].unsqueeze(2).broadcast_to(
                        [P, NBc, SLOTS])
                    iota_b = iota_t[:].unsqueeze(1).broadcast_to([P, NBc, SLOTS])
                    nc.vector.tensor_tensor(out=eq[:, :NBc, :], in0=slot_b,
                                            in1=iota_b, op=OP.is_equal)
                    m4c = mpool.tile([P, KBMAX * N_SHARDS, SLOTS], F8, tag="m4")
                    w_b = sw[:, 1, :NBc].unsqueeze(2).broadcast_to(
                        [P, NBc, SLOTS])
                    nc.vector.tensor_tensor(out=m4c[:, :NBc, :],
                                            in0=eq[:, :NBc, :], in1=w_b,
                                            op=OP.mult)
                    GRP = 8  # bins per PSUM bank (8 x 64 f32 = one 2KB bank)
                    for k in range(kb_ch):
                        b = bin0 + k
                        if k % GRP == 0:
                            pf = psf.tile([P, GRP * SLOTS], F32, tag="pf")
                        col = (k % GRP) * SLOTS
                        for s in range(N_SHARDS):
                            nc.tensor.matmul(
                                out=pf[:, col:col + SLOTS],
                                lhsT=gts[s][:, k, :],
                                rhs=m4c[:, k * N_SHARDS + s, :],
                                start=(s == 0), stop=(s == N_SHARDS - 1))
                        if k % GRP == GRP - 1 or k == kb_ch - 1:
                            ncols = (k % GRP) + 1
                            lo_b = b - ncols + 1
                            nc.scalar.copy(
                                out=featsT[:, lo_b * SLOTS:(b + 1) * SLOTS],
                                in_=pf[:, :ncols * SLOTS])
                    bin0 += kb_ch
                emit_mlp(mlp_done, nslots // mlp_chunk)
            nc.sync.dma_start(out=out_d[:], in_=out_sb[:])
    nc.compile()
    return nc


# ---------------------------------------------------------------- entry point

def _host_fallback(out, segs, x, node_idx, hyperedge_idx, W1, b1, W2, b2):
    for s in segs:
        rows = x[node_idx[hyperedge_idx == s]]
        feats = rows.mean(axis=0) if len(rows) else np.zeros(IN_DIM, np.float32)
        h = np.maximum(np.einsum("d,hdk->hk", feats, W1) + b1, 0.0)
        alpha = np.einsum("hk,hk->h", h, W2) + b2
        w = 1.0 / (1.0 + np.exp(-np.clip(alpha, -5, 5)))
        out[s] = w.mean() * 0.9 + 0.1


def kernel(x, node_idx, hyperedge_idx, W1, b1, W2, b2):
    x = np.asarray(x, np.float32)
    node_idx = np.asarray(node_idx)
    hyperedge_idx = np.asarray(hyperedge_idx)
    W1 = np.asarray(W1, np.float32); b1 = np.asarray(b1, np.float32)
    W2 = np.asarray(W2, np.float32); b2 = np.asarray(b2, np.float32)

    packed = _pack(node_idx, hyperedge_idx)
    m = packed["meta"]
    consts = _make_mlp_consts(W1, b1, W2, b2)
    nc = build_nc(m["nbins"], N_CORES)
    in_maps = [_make_in_map(c, x, packed, consts) for c in range(N_CORES)]
    res = run_bass_kernel_spmd(nc, in_maps, list(range(N_CORES)))

    out = np.full(NUM_HYPEREDGES, np.nan, dtype=np.float32)
    om = packed["out_map"].reshape(N_CORES, -1)
    for c in range(N_CORES):
        core_out = res.results[c]["out"].reshape(-1)
        v = om[c] >= 0
        out[om[c][v]] = core_out[v]
    if len(packed["fallback"]):
        _host_fallback(out, packed["fallback"], x, node_idx, hyperedge_idx,
                       W1, b1, W2, b2)
    assert not np.isnan(out).any()
    return out


# revision 6
# speedup vs baseline: 1.0108x; 1.0108x over previous
"""TRN2 Bass kernel for nn_MultiHeadHyperedgeAttention.

Pipeline (8 NeuronCores, hyperedge-sharded, no collectives):
  host: sort edges by hyperedge; first-fit-decreasing bin packing of
        segments into bins of <=64 slots with <=128 edges per node-shard
        (4 shards of 25000 rows so gather indices fit int16); build
        per-bin wrapped gather-index tables and compact slot/weight
        tables; x converted to fp8(e4m3), rows padded to 256B stride.
  dev:  per chunk (16 bins; tail chunk may be 8):
        - dma_gather fp8 rows (128B descriptors, stride 256B) on 4
          SWDGE queues, one per node-shard
        - build the scaled one-hot M on-device (DVE is_equal/mult with
          3D broadcast APs against an iota constant)
        - per-bin matmuls G^T @ M accumulated over shards in PSUM
          (fp8 x fp8 -> f32), PSUM -> f16 featsT via ACT copies
        - MLP for the previous chunk's slots issued before this chunk's
          segment matmuls (3 matmuls + ACT relu/sigmoid + DVE clip),
          f16 weights, f32 PSUM
  host: scatter slot outputs back to the [50000] output.
"""
import numpy as np

import concourse.bass as bass
import concourse.tile as tile
from concourse import ap_utils, bacc, mybir
from concourse.library_config import mlp as mlp_lib
from concourse.bass_utils import run_bass_kernel_spmd

NUM_NODES = 100000
NUM_HYPEREDGES = 50000
IN_DIM = 128
NUM_HEADS = 8
N_CORES = 8
N_SHARDS = 4
SHARD = NUM_NODES // N_SHARDS      # 25000 rows -> int16-safe gather indices
SLOTS = 64                         # segment slots per bin
BINCAP = 128                       # per-shard edge capacity per bin
KBMAX = 16                         # max bins per chunk
PAD_SLOT = 999.0
P = 128
D = IN_DIM
F32 = mybir.dt.float32
F16 = mybir.dt.float16
F8 = mybir.dt.float8e4
I16 = mybir.dt.int16
XPAD = 256                         # fp8 row stride in bytes (= 256B units)
NP_F8 = mybir.dt.np(F8)
AF = mybir.ActivationFunctionType
OP = mybir.AluOpType
SIG_LO = 1.0 / (1.0 + np.exp(5.0))
SIG_HI = 1.0 / (1.0 + np.exp(-5.0))


# ---------------------------------------------------------------- host packing

def _pack(node_idx, hyperedge_idx, binmul=8):
    node_idx = np.asarray(node_idx, dtype=np.int64)
    hyperedge_idx = np.asarray(hyperedge_idx, dtype=np.int64)
    counts = np.bincount(hyperedge_idx, minlength=NUM_HYPEREDGES)
    inv_cnt = 1.0 / np.maximum(counts, 1).astype(np.float64)

    shard_of_edge = node_idx // SHARD
    order = np.lexsort((node_idx, shard_of_edge, hyperedge_idx))
    e_node = node_idx[order]
    e_shard = shard_of_edge[order]

    cnt_ss = np.zeros((NUM_HYPEREDGES, N_SHARDS), dtype=np.int64)
    np.add.at(cnt_ss, (hyperedge_idx, shard_of_edge), 1)
    seg_starts = np.zeros(NUM_HYPEREDGES + 1, dtype=np.int64)
    seg_starts[1:] = np.cumsum(counts)

    # segments whose per-shard edge count exceeds one bin go to the host
    # fallback path (never happens for the target distribution)
    fallback = np.where(cnt_ss.max(axis=1) > BINCAP)[0]
    fb = set(fallback.tolist())

    seg_per_core = NUM_HYPEREDGES // N_CORES
    cores = []
    for c in range(N_CORES):
        s0, s1 = c * seg_per_core, (c + 1) * seg_per_core
        segs_c = [s for s in range(s0, s1) if s not in fb]
        # first-fit-decreasing vector bin packing: capacity BINCAP per
        # shard, <= SLOTS segments per bin (~97% fill)
        order_c = sorted(segs_c, key=lambda s: -int(cnt_ss[s].max()))
        bin_cnt = np.zeros((0, N_SHARDS), dtype=np.int64)
        bin_segs = []
        bin_free = np.zeros(0, dtype=np.int64)
        for s in order_c:
            csm = cnt_ss[s]
            fits = np.where((bin_free > 0) &
                            ((bin_cnt + csm) <= BINCAP).all(axis=1))[0]
            if len(fits):
                i = int(fits[0])
                bin_cnt[i] += csm
                bin_segs[i].append(s)
                bin_free[i] -= 1
            else:
                bin_cnt = np.concatenate([bin_cnt, csm[None]], axis=0)
                bin_segs.append([s])
                bin_free = np.concatenate([bin_free, [SLOTS - 1]])
        cores.append([(bin_segs[i], bin_cnt[i]) for i in range(len(bin_segs))])

    nbins = max(len(b) for b in cores)
    nbins = -(-nbins // binmul) * binmul

    idx16 = np.zeros((N_CORES, N_SHARDS, nbins, BINCAP), dtype=np.int16)
    slotf = np.full((N_CORES, nbins, BINCAP, N_SHARDS), PAD_SLOT, dtype=np.float32)
    wf = np.zeros((N_CORES, nbins, BINCAP, N_SHARDS), dtype=np.float32)
    out_map = np.full((N_CORES, nbins, SLOTS), -1, dtype=np.int64)

    for c in range(N_CORES):
        for b, (segs, _cnt) in enumerate(cores[c]):
            out_map[c, b, :len(segs)] = segs
            pos = np.zeros(N_SHARDS, dtype=np.int64)
            for sl, s in enumerate(segs):
                e0, e1 = seg_starts[s], seg_starts[s + 1]
                nodes = e_node[e0:e1]
                shards = e_shard[e0:e1]
                for sh in range(N_SHARDS):
                    msk = shards == sh
                    k = int(msk.sum())
                    if k == 0:
                        continue
                    p0 = pos[sh]
                    idx16[c, sh, b, p0:p0 + k] = (nodes[msk] - sh * SHARD).astype(np.int16)
                    slotf[c, b, p0:p0 + k, sh] = sl
                    wf[c, b, p0:p0 + k, sh] = inv_cnt[s]
                    pos[sh] += k
            # sort each shard's 128 positions by node id for HBM locality
            for sh in range(N_SHARDS):
                o = np.argsort(idx16[c, sh, b], kind="stable")
                idx16[c, sh, b] = idx16[c, sh, b][o]
                slotf[c, b, :, sh] = slotf[c, b, o, sh]
                wf[c, b, :, sh] = wf[c, b, o, sh]

    # per-bin wrapped idx blocks (idx i -> partition i%16, col i//16 within
    # any bin-aligned window): [P, N_SHARDS, nbins*8]
    IW = BINCAP // 16
    gidx = np.zeros((N_CORES, P, N_SHARDS, nbins * IW), dtype=np.int16)
    for c in range(N_CORES):
        for sh in range(N_SHARDS):
            w = idx16[c, sh].reshape(nbins, IW, 16).transpose(0, 2, 1)
            blk = w.transpose(1, 0, 2).reshape(16, nbins * IW)
            gidx[c, :, sh, :] = np.tile(blk, (8, 1))

    # compact slot/weight tables: [P, 2, nbins*N_SHARDS], col = b*N_SHARDS+s
    slotw = np.zeros((N_CORES, P, 2, nbins * N_SHARDS), dtype=np.float16)
    slotw[:, :, 0, :] = slotf.transpose(0, 2, 1, 3).reshape(N_CORES, P, -1)
    slotw[:, :, 1, :] = wf.transpose(0, 2, 1, 3).reshape(N_CORES, P, -1)

    meta = dict(nbins=nbins, nslots=nbins * SLOTS)
    return dict(gidx=gidx, slotw=slotw, out_map=out_map,
                fallback=fallback, meta=meta)


def _make_mlp_consts(W1, b1, W2, b2):
    W1 = np.asarray(W1, np.float32); b1 = np.asarray(b1, np.float32)
    W2 = np.asarray(W2, np.float32); b2 = np.asarray(b2, np.float32)
    H, Din, K = W1.shape
    w1cat = np.ascontiguousarray(W1.transpose(1, 0, 2).reshape(Din, H * K))
    w2blk = np.zeros((H * K, H), np.float32)
    for h in range(H):
        w2blk[h * K:(h + 1) * K, h] = W2[h]
    iota64 = np.broadcast_to(np.arange(SLOTS, dtype=np.float16), (P, SLOTS)).copy()
    return dict(w1cat=w1cat.astype(np.float16), b1cat=b1.reshape(H * K, 1),
                w2blk=w2blk.astype(np.float16), b2col=b2.reshape(H, 1),
                meanw=np.full((H, 1), 0.9 / H, np.float16),
                iota64=iota64)


def _make_in_map(core, x, packed, consts):
    im = {
        "gidx": packed["gidx"][core],
        "slotw": packed["slotw"][core],
        **consts,
    }
    for s in range(N_SHARDS):
        xp = np.zeros((SHARD, XPAD), dtype=NP_F8)
        xp[:, :D] = x[s * SHARD:(s + 1) * SHARD].astype(NP_F8)
        im[f"xs{s}"] = xp
    return im


# ---------------------------------------------------------------- device kernel

def _dma_gather_raw(g, out_ap, in_ap, idxs_ap, num_idxs, elem_size, elem_step,
                    queue_num):
    """dma_gather minus the vestigial elem_size_bytes%256 assert
    (non-transpose, HBM source). The descriptor stride field still
    requires elem_step bytes to be a multiple of 256."""
    g._assert_queue_num(queue_num)
    assert idxs_ap.dtype == mybir.dt.int16
    assert in_ap.dtype == out_ap.dtype
    assert in_ap.space == bass.MemorySpace.DRAM
    assert idxs_ap.space == bass.MemorySpace.SBUF
    assert out_ap.space == bass.MemorySpace.SBUF
    assert ap_utils.ap_is_contiguous(out_ap.ap[1:])
    assert ap_utils.ap_is_contiguous(idxs_ap.ap[1:])
    assert in_ap.ap[-1][1] == out_ap.ap[-1][1] == elem_size
    assert out_ap.ap[0][1] * out_ap.ap[1][1] == -(-num_idxs // 128) * 128
    assert in_ap.ap[0][0] == elem_step
    stride_bytes = elem_step * mybir.dt.size(in_ap.dtype)
    assert stride_bytes % 256 == 0
    stride_bytes_256 = stride_bytes // 256
    assert stride_bytes_256 < 256
    _in_ap = g.lower_ap_dma(in_ap, for_custom_bir_dma=True)
    _idxs_ap = g.lower_ap(idxs_ap)
    _out_ap = g.lower_ap(out_ap)
    return g.add_instruction(
        mybir.InstDMAGatherAnt(
            name=g.bass.get_next_instruction_name(),
            ins=[*_in_ap, _idxs_ap, g.lower_val_access(g.to_reg(num_idxs))],
            outs=[_out_ap],
            transpose=False,
            num_idxs=num_idxs,
            elem_size=elem_size,
            stride_bytes_256=stride_bytes_256,
            gen_mode=0,
            single_packet=False,
            queue_num=queue_num,
            sbuf_tokens_per_rank=0,
            sbuf_free_dim_per_rank=0,
            sbuf_free_dim_pad_per_rank=0,
            sbuf_byte_offset=0,
        )
    )


def build_nc(nbins, n_cores, mlp_chunk=512, repeat=1, gbufs=16):
    nslots = nbins * SLOTS
    assert nslots % mlp_chunk == 0
    IW = BINCAP // 16
    chunk_plan = [KBMAX] * (nbins // KBMAX)
    if nbins % KBMAX:
        chunk_plan.append(nbins % KBMAX)
    nc = bacc.Bacc("TRN2", target_bir_lowering=False, debug=False,
                   num_devices=n_cores, num_swdge_queues=4)
    xs = [nc.dram_tensor(f"xs{s}", [SHARD, XPAD], F8, kind="ExternalInput").ap()
          for s in range(N_SHARDS)]
    gidx = nc.dram_tensor("gidx", [P, N_SHARDS, nbins * IW], I16,
                          kind="ExternalInput").ap()
    slotw_d = nc.dram_tensor("slotw", [P, 2, nbins * N_SHARDS], F16,
                             kind="ExternalInput").ap()
    w1_d = nc.dram_tensor("w1cat", [D, 64], F16, kind="ExternalInput").ap()
    b1_d = nc.dram_tensor("b1cat", [64, 1], F32, kind="ExternalInput").ap()
    w2_d = nc.dram_tensor("w2blk", [64, 8], F16, kind="ExternalInput").ap()
    b2_d = nc.dram_tensor("b2col", [8, 1], F32, kind="ExternalInput").ap()
    mean_d = nc.dram_tensor("meanw", [8, 1], F16, kind="ExternalInput").ap()
    iota_d = nc.dram_tensor("iota64", [P, SLOTS], F16, kind="ExternalInput").ap()
    out_d = nc.dram_tensor("out", [1, nslots], F32, kind="ExternalOutput").ap()

    with tile.TileContext(nc) as tc:
        with (
            tc.tile_pool(name="consts", bufs=1) as cpool,
            tc.tile_pool(name="idx", bufs=3) as ipool,
            tc.tile_pool(name="sw", bufs=3) as spool,
            tc.tile_pool(name="g", bufs=gbufs) as gpool,
            tc.tile_pool(name="m4w", bufs=2) as mpool,
            tc.tile_pool(name="eq", bufs=2) as epool,
            tc.tile_pool(name="feats", bufs=1) as fpool,
            tc.tile_pool(name="mlptmp", bufs=3) as tpool,
            tc.tile_pool(name="outp", bufs=1) as opool,
            tc.tile_pool(name="psf", bufs=3, space="PSUM") as psf,
            tc.tile_pool(name="psh", bufs=2, space="PSUM") as psh,
            tc.tile_pool(name="psa", bufs=1, space="PSUM") as psa,
            tc.tile_pool(name="pso", bufs=1, space="PSUM") as pso,
        ):
            nc.gpsimd.load_library(mlp_lib)
            w1_t = cpool.tile([D, 64], F16)
            nc.sync.dma_start(out=w1_t[:], in_=w1_d[:])
            b1_t = cpool.tile([64, 1], F32)
            nc.sync.dma_start(out=b1_t[:], in_=b1_d[:])
            w2_t = cpool.tile([64, 8], F16)
            nc.sync.dma_start(out=w2_t[:], in_=w2_d[:])
            b2_t = cpool.tile([8, 1], F32)
            nc.sync.dma_start(out=b2_t[:], in_=b2_d[:])
            mean_t = cpool.tile([8, 1], F16)
            nc.sync.dma_start(out=mean_t[:], in_=mean_d[:])
            iota_t = cpool.tile([P, SLOTS], F16)
            nc.sync.dma_start(out=iota_t[:], in_=iota_d[:])

            featsT = fpool.tile([P, nslots], F16)
            out_sb = opool.tile([1, nslots], F32)

            def emit_mlp(lo, hi):
                for j in range(lo, hi):
                    cols = slice(j * mlp_chunk, (j + 1) * mlp_chunk)
                    ph = psh.tile([64, mlp_chunk], F32, tag="ph")
                    nc.tensor.matmul(out=ph[:], lhsT=w1_t[:],
                                     rhs=featsT[:, cols], start=True, stop=True)
                    hr = tpool.tile([64, mlp_chunk], F16, tag="hr")
                    nc.scalar.activation(out=hr[:], in_=ph[:], func=AF.Relu,
                                         bias=b1_t[:])
                    pa = psa.tile([8, mlp_chunk], F32, tag="pa")
                    nc.tensor.matmul(out=pa[:], lhsT=w2_t[:], rhs=hr[:],
                                     start=True, stop=True)
                    sg = tpool.tile([8, mlp_chunk], F16, tag="sg")
                    nc.scalar.activation(out=sg[:], in_=pa[:], func=AF.Sigmoid,
                                         bias=b2_t[:])
                    nc.vector.tensor_scalar(out=sg[:], in0=sg[:],
                                            scalar1=float(SIG_LO),
                                            scalar2=float(SIG_HI),
                                            op0=OP.max, op1=OP.min)
                    po = pso.tile([1, mlp_chunk], F32, tag="po")
                    nc.tensor.matmul(out=po[:], lhsT=mean_t[:], rhs=sg[:],
                                     start=True, stop=True)
                    nc.scalar.activation(out=out_sb[:, cols], in_=po[:],
                                         func=AF.Copy, bias=0.1)

            for _r in range(repeat):
                bin0 = 0
                mlp_done = 0
                for kb_ch in chunk_plan:
                    # MLP for completed slots (previous chunks) goes first so
                    # PE isn't head-of-line blocked behind gather-dependent
                    # segment matmuls
                    avail = (bin0 * SLOTS) // mlp_chunk
                    emit_mlp(mlp_done, avail)
                    mlp_done = avail
                    nid = kb_ch * BINCAP
                    it = ipool.tile([P, N_SHARDS, KBMAX * IW], I16, tag="idx")
                    nc.sync.dma_start(
                        out=it[:, :, :kb_ch * IW],
                        in_=gidx[:, :, bin0 * IW:(bin0 + kb_ch) * IW])
                    sw = spool.tile([P, 2, KBMAX * N_SHARDS], F16, tag="sw")
                    nc.sync.dma_start(
                        out=sw[:, :, :kb_ch * N_SHARDS],
                        in_=slotw_d[:, :, bin0 * N_SHARDS:(bin0 + kb_ch) * N_SHARDS])
                    gts = []
                    for s in range(N_SHARDS):
                        G = gpool.tile([P, KBMAX, D], F8, tag="G")
                        _dma_gather_raw(
                            nc.gpsimd, G[:, :kb_ch, :], xs[s][:, 0:D],
                            it[:, s, :kb_ch * IW],
                            nid, D, XPAD, queue_num=s)
                        gts.append(G)
                    # build the scaled one-hot on-device:
                    #   m4[p, j, t] = (slot[p, j] == t) * w[p, j]
                    NBc = kb_ch * N_SHARDS
                    eq = epool.tile([P, KBMAX * N_SHARDS, SLOTS], F16, tag="eq")
                    slot_b = sw[:, 0, :NBc].unsqueeze(2).broadcast_to(
                        [P, NBc, SLOTS])
                    iota_b = iota_t[:].unsqueeze(1).broadcast_to([P, NBc, SLOTS])
                    nc.vector.tensor_tensor(out=eq[:, :NBc, :], in0=slot_b,
                                            in1=iota_b, op=OP.is_equal)
                    m4c = mpool.tile([P, KBMAX * N_SHARDS, SLOTS], F8, tag="m4")
                    w_b = sw[:, 1, :NBc].unsqueeze(2).broadcast_to(
                        [P, NBc, SLOTS])
                    nc.vector.tensor_tensor(out=m4c[:, :NBc, :],
                                            in0=eq[:, :NBc, :], in1=w_b,
                                            op=OP.mult)
                    GRP = 8  # bins per PSUM bank (8 x 64 f32 = one 2KB bank)
                    for k in range(kb_ch):
                        b = bin0 + k
                        if k % GRP == 0:
                            pf = psf.tile([P, GRP * SLOTS], F32, tag="pf")
                        col = (k % GRP) * SLOTS
                        for s in range(N_SHARDS):
                            nc.tensor.matmul(
                                out=pf[:, col:col + SLOTS],
                                lhsT=gts[s][:, k, :],
                                rhs=m4c[:, k * N_SHARDS + s, :],
                                start=(s == 0), stop=(s == N_SHARDS - 1))
                        if k % GRP == GRP - 1 or k == kb_ch - 1:
                            ncols = (k % GRP) + 1
                            lo_b = b - ncols + 1
                            nc.scalar.copy(
                                out=featsT[:, lo_b * SLOTS:(b + 1) * SLOTS],
                                in_=pf[:, :ncols * SLOTS])
                    bin0 += kb_ch
                emit_mlp(mlp_done, nslots // mlp_chunk)
            nc.sync.dma_start(out=out_d[:], in_=out_sb[:])
    nc.compile()
    return nc


# ---------------------------------------------------------------- entry point

def _host_fallback(out, segs, x, node_idx, hyperedge_idx, W1, b1, W2, b2):
    for s in segs:
        rows = x[node_idx[hyperedge_idx == s]]
        feats = rows.mean(axis=0) if len(rows) else np.zeros(IN_DIM, np.float32)
        h = np.maximum(np.einsum("d,hdk->hk", feats, W1) + b1, 0.0)
        alpha = np.einsum("hk,hk->h", h, W2) + b2
        w = 1.0 / (1.0 + np.exp(-np.clip(alpha, -5, 5)))
        out[s] = w.mean() * 0.9 + 0.1


def kernel(x, node_idx, hyperedge_idx, W1, b1, W2, b2):
    x = np.asarray(x, np.float32)
    node_idx = np.asarray(node_idx)
    hyperedge_idx = np.asarray(hyperedge_idx)
    W1 = np.asarray(W1, np.float32); b1 = np.asarray(b1, np.float32)
    W2 = np.asarray(W2, np.float32); b2 = np.asarray(b2, np.float32)

    packed = _pack(node_idx, hyperedge_idx)
    m = packed["meta"]
    consts = _make_mlp_consts(W1, b1, W2, b2)
    nc = build_nc(m["nbins"], N_CORES)
    in_maps = [_make_in_map(c, x, packed, consts) for c in range(N_CORES)]
    res = run_bass_kernel_spmd(nc, in_maps, list(range(N_CORES)))

    out = np.full(NUM_HYPEREDGES, np.nan, dtype=np.float32)
    om = packed["out_map"].reshape(N_CORES, -1)
    for c in range(N_CORES):
        core_out = res.results[c]["out"].reshape(-1)
        v = om[c] >= 0
        out[om[c][v]] = core_out[v]
    if len(packed["fallback"]):
        _host_fallback(out, packed["fallback"], x, node_idx, hyperedge_idx,
                       W1, b1, W2, b2)
    assert not np.isnan(out).any()
    return out


# revision 7
# speedup vs baseline: 1.0396x; 1.0285x over previous
"""TRN2 Bass kernel for nn_MultiHeadHyperedgeAttention.

Pipeline (8 NeuronCores, hyperedge-sharded, no collectives):
  host: sort edges by hyperedge; first-fit-decreasing bin packing of
        segments into bins of <=64 slots with <=128 edges per node-shard
        (4 shards of 25000 rows so gather indices fit int16); build
        per-bin wrapped gather-index tables and compact slot/weight
        tables; x converted to fp8(e4m3), rows padded to 256B stride.
  dev:  per chunk (16 bins; tail chunk may be 8):
        - dma_gather fp8 rows (128B descriptors, stride 256B) on 4
          SWDGE queues, one per node-shard
        - build the scaled one-hot M on-device (DVE is_equal/mult with
          3D broadcast APs against an iota constant)
        - per-bin matmuls G^T @ M accumulated over shards in PSUM
          (fp8 x fp8 -> f32), PSUM -> f16 featsT via ACT copies
        - MLP for the previous chunk's slots issued before this chunk's
          segment matmuls (3 matmuls + ACT relu/sigmoid + DVE clip),
          f16 weights, f32 PSUM
  host: scatter slot outputs back to the [50000] output.
"""
import numpy as np

import concourse.bass as bass
import concourse.tile as tile
from concourse import ap_utils, bacc, mybir
from concourse.library_config import mlp as mlp_lib
from concourse.bass_utils import run_bass_kernel_spmd

NUM_NODES = 100000
NUM_HYPEREDGES = 50000
IN_DIM = 128
NUM_HEADS = 8
N_CORES = 8
N_SHARDS = 4
SHARD = NUM_NODES // N_SHARDS      # 25000 rows -> int16-safe gather indices
SLOTS = 64                         # segment slots per bin
BINCAP = 128                       # per-shard edge capacity per bin
KBMAX = 24                         # max bins per chunk (3072-desc gather calls)
PAD_SLOT = 999.0
P = 128
D = IN_DIM
F32 = mybir.dt.float32
F16 = mybir.dt.float16
F8 = mybir.dt.float8e4
I16 = mybir.dt.int16
XPAD = 256                         # fp8 row stride in bytes (= 256B units)
NP_F8 = mybir.dt.np(F8)
AF = mybir.ActivationFunctionType
OP = mybir.AluOpType
SIG_LO = 1.0 / (1.0 + np.exp(5.0))
SIG_HI = 1.0 / (1.0 + np.exp(-5.0))


# ---------------------------------------------------------------- host packing

def _pack(node_idx, hyperedge_idx, binmul=8):
    node_idx = np.asarray(node_idx, dtype=np.int64)
    hyperedge_idx = np.asarray(hyperedge_idx, dtype=np.int64)
    counts = np.bincount(hyperedge_idx, minlength=NUM_HYPEREDGES)
    inv_cnt = 1.0 / np.maximum(counts, 1).astype(np.float64)

    shard_of_edge = node_idx // SHARD
    order = np.lexsort((node_idx, shard_of_edge, hyperedge_idx))
    e_node = node_idx[order]
    e_shard = shard_of_edge[order]

    cnt_ss = np.zeros((NUM_HYPEREDGES, N_SHARDS), dtype=np.int64)
    np.add.at(cnt_ss, (hyperedge_idx, shard_of_edge), 1)
    seg_starts = np.zeros(NUM_HYPEREDGES + 1, dtype=np.int64)
    seg_starts[1:] = np.cumsum(counts)

    # segments whose per-shard edge count exceeds one bin go to the host
    # fallback path (never happens for the target distribution)
    fallback = np.where(cnt_ss.max(axis=1) > BINCAP)[0]
    fb = set(fallback.tolist())

    seg_per_core = NUM_HYPEREDGES // N_CORES
    cores = []
    for c in range(N_CORES):
        s0, s1 = c * seg_per_core, (c + 1) * seg_per_core
        segs_c = [s for s in range(s0, s1) if s not in fb]
        # first-fit-decreasing vector bin packing: capacity BINCAP per
        # shard, <= SLOTS segments per bin (~97% fill)
        order_c = sorted(segs_c, key=lambda s: -int(cnt_ss[s].max()))
        bin_cnt = np.zeros((0, N_SHARDS), dtype=np.int64)
        bin_segs = []
        bin_free = np.zeros(0, dtype=np.int64)
        for s in order_c:
            csm = cnt_ss[s]
            fits = np.where((bin_free > 0) &
                            ((bin_cnt + csm) <= BINCAP).all(axis=1))[0]
            if len(fits):
                i = int(fits[0])
                bin_cnt[i] += csm
                bin_segs[i].append(s)
                bin_free[i] -= 1
            else:
                bin_cnt = np.concatenate([bin_cnt, csm[None]], axis=0)
                bin_segs.append([s])
                bin_free = np.concatenate([bin_free, [SLOTS - 1]])
        cores.append([(bin_segs[i], bin_cnt[i]) for i in range(len(bin_segs))])

    nbins = max(len(b) for b in cores)
    nbins = -(-nbins // binmul) * binmul

    idx16 = np.zeros((N_CORES, N_SHARDS, nbins, BINCAP), dtype=np.int16)
    slotf = np.full((N_CORES, nbins, BINCAP, N_SHARDS), PAD_SLOT, dtype=np.float32)
    wf = np.zeros((N_CORES, nbins, BINCAP, N_SHARDS), dtype=np.float32)
    out_map = np.full((N_CORES, nbins, SLOTS), -1, dtype=np.int64)

    for c in range(N_CORES):
        for b, (segs, _cnt) in enumerate(cores[c]):
            out_map[c, b, :len(segs)] = segs
            pos = np.zeros(N_SHARDS, dtype=np.int64)
            for sl, s in enumerate(segs):
                e0, e1 = seg_starts[s], seg_starts[s + 1]
                nodes = e_node[e0:e1]
                shards = e_shard[e0:e1]
                for sh in range(N_SHARDS):
                    msk = shards == sh
                    k = int(msk.sum())
                    if k == 0:
                        continue
                    p0 = pos[sh]
                    idx16[c, sh, b, p0:p0 + k] = (nodes[msk] - sh * SHARD).astype(np.int16)
                    slotf[c, b, p0:p0 + k, sh] = sl
                    wf[c, b, p0:p0 + k, sh] = inv_cnt[s]
                    pos[sh] += k
            # sort each shard's 128 positions by node id for HBM locality
            for sh in range(N_SHARDS):
                o = np.argsort(idx16[c, sh, b], kind="stable")
                idx16[c, sh, b] = idx16[c, sh, b][o]
                slotf[c, b, :, sh] = slotf[c, b, o, sh]
                wf[c, b, :, sh] = wf[c, b, o, sh]

    # per-bin wrapped idx blocks (idx i -> partition i%16, col i//16 within
    # any bin-aligned window): [P, N_SHARDS, nbins*8]
    IW = BINCAP // 16
    gidx = np.zeros((N_CORES, P, N_SHARDS, nbins * IW), dtype=np.int16)
    for c in range(N_CORES):
        for sh in range(N_SHARDS):
            w = idx16[c, sh].reshape(nbins, IW, 16).transpose(0, 2, 1)
            blk = w.transpose(1, 0, 2).reshape(16, nbins * IW)
            gidx[c, :, sh, :] = np.tile(blk, (8, 1))

    # compact slot/weight tables: [P, 2, nbins*N_SHARDS], col = b*N_SHARDS+s
    slotw = np.zeros((N_CORES, P, 2, nbins * N_SHARDS), dtype=np.float16)
    slotw[:, :, 0, :] = slotf.transpose(0, 2, 1, 3).reshape(N_CORES, P, -1)
    slotw[:, :, 1, :] = wf.transpose(0, 2, 1, 3).reshape(N_CORES, P, -1)

    meta = dict(nbins=nbins, nslots=nbins * SLOTS)
    return dict(gidx=gidx, slotw=slotw, out_map=out_map,
                fallback=fallback, meta=meta)


def _make_mlp_consts(W1, b1, W2, b2):
    W1 = np.asarray(W1, np.float32); b1 = np.asarray(b1, np.float32)
    W2 = np.asarray(W2, np.float32); b2 = np.asarray(b2, np.float32)
    H, Din, K = W1.shape
    w1cat = np.ascontiguousarray(W1.transpose(1, 0, 2).reshape(Din, H * K))
    w2blk = np.zeros((H * K, H), np.float32)
    for h in range(H):
        w2blk[h * K:(h + 1) * K, h] = W2[h]
    iota64 = np.broadcast_to(np.arange(SLOTS, dtype=np.float16), (P, SLOTS)).copy()
    return dict(w1cat=w1cat.astype(np.float16), b1cat=b1.reshape(H * K, 1),
                w2blk=w2blk.astype(np.float16), b2col=b2.reshape(H, 1),
                meanw=np.full((H, 1), 0.9 / H, np.float16),
                iota64=iota64)


def _make_in_map(core, x, packed, consts):
    im = {
        "gidx": packed["gidx"][core],
        "slotw": packed["slotw"][core],
        **consts,
    }
    for s in range(N_SHARDS):
        xp = np.zeros((SHARD, XPAD), dtype=NP_F8)
        xp[:, :D] = x[s * SHARD:(s + 1) * SHARD].astype(NP_F8)
        im[f"xs{s}"] = xp
    return im


# ---------------------------------------------------------------- device kernel

def _dma_gather_raw(g, out_ap, in_ap, idxs_ap, num_idxs, elem_size, elem_step,
                    queue_num):
    """dma_gather minus the vestigial elem_size_bytes%256 assert
    (non-transpose, HBM source). The descriptor stride field still
    requires elem_step bytes to be a multiple of 256."""
    g._assert_queue_num(queue_num)
    assert idxs_ap.dtype == mybir.dt.int16
    assert in_ap.dtype == out_ap.dtype
    assert in_ap.space == bass.MemorySpace.DRAM
    assert idxs_ap.space == bass.MemorySpace.SBUF
    assert out_ap.space == bass.MemorySpace.SBUF
    assert ap_utils.ap_is_contiguous(out_ap.ap[1:])
    assert ap_utils.ap_is_contiguous(idxs_ap.ap[1:])
    assert in_ap.ap[-1][1] == out_ap.ap[-1][1] == elem_size
    assert out_ap.ap[0][1] * out_ap.ap[1][1] == -(-num_idxs // 128) * 128
    assert in_ap.ap[0][0] == elem_step
    stride_bytes = elem_step * mybir.dt.size(in_ap.dtype)
    assert stride_bytes % 256 == 0
    stride_bytes_256 = stride_bytes // 256
    assert stride_bytes_256 < 256
    _in_ap = g.lower_ap_dma(in_ap, for_custom_bir_dma=True)
    _idxs_ap = g.lower_ap(idxs_ap)
    _out_ap = g.lower_ap(out_ap)
    return g.add_instruction(
        mybir.InstDMAGatherAnt(
            name=g.bass.get_next_instruction_name(),
            ins=[*_in_ap, _idxs_ap, g.lower_val_access(g.to_reg(num_idxs))],
            outs=[_out_ap],
            transpose=False,
            num_idxs=num_idxs,
            elem_size=elem_size,
            stride_bytes_256=stride_bytes_256,
            gen_mode=0,
            single_packet=False,
            queue_num=queue_num,
            sbuf_tokens_per_rank=0,
            sbuf_free_dim_per_rank=0,
            sbuf_free_dim_pad_per_rank=0,
            sbuf_byte_offset=0,
        )
    )


def build_nc(nbins, n_cores, mlp_chunk=512, repeat=1, gbufs=12):
    nslots = nbins * SLOTS
    assert nslots % mlp_chunk == 0
    IW = BINCAP // 16
    chunk_plan = [KBMAX] * (nbins // KBMAX)
    if nbins % KBMAX:
        chunk_plan.append(nbins % KBMAX)
    nc = bacc.Bacc("TRN2", target_bir_lowering=False, debug=False,
                   num_devices=n_cores, num_swdge_queues=4)
    xs = [nc.dram_tensor(f"xs{s}", [SHARD, XPAD], F8, kind="ExternalInput").ap()
          for s in range(N_SHARDS)]
    gidx = nc.dram_tensor("gidx", [P, N_SHARDS, nbins * IW], I16,
                          kind="ExternalInput").ap()
    slotw_d = nc.dram_tensor("slotw", [P, 2, nbins * N_SHARDS], F16,
                             kind="ExternalInput").ap()
    w1_d = nc.dram_tensor("w1cat", [D, 64], F16, kind="ExternalInput").ap()
    b1_d = nc.dram_tensor("b1cat", [64, 1], F32, kind="ExternalInput").ap()
    w2_d = nc.dram_tensor("w2blk", [64, 8], F16, kind="ExternalInput").ap()
    b2_d = nc.dram_tensor("b2col", [8, 1], F32, kind="ExternalInput").ap()
    mean_d = nc.dram_tensor("meanw", [8, 1], F16, kind="ExternalInput").ap()
    iota_d = nc.dram_tensor("iota64", [P, SLOTS], F16, kind="ExternalInput").ap()
    out_d = nc.dram_tensor("out", [1, nslots], F32, kind="ExternalOutput").ap()

    with tile.TileContext(nc) as tc:
        with (
            tc.tile_pool(name="consts", bufs=1) as cpool,
            tc.tile_pool(name="idx", bufs=3) as ipool,
            tc.tile_pool(name="sw", bufs=3) as spool,
            tc.tile_pool(name="g", bufs=gbufs) as gpool,
            tc.tile_pool(name="m4w", bufs=2) as mpool,
            tc.tile_pool(name="eq", bufs=2) as epool,
            tc.tile_pool(name="feats", bufs=1) as fpool,
            tc.tile_pool(name="mlptmp", bufs=3) as tpool,
            tc.tile_pool(name="outp", bufs=1) as opool,
            tc.tile_pool(name="psf", bufs=3, space="PSUM") as psf,
            tc.tile_pool(name="psh", bufs=2, space="PSUM") as psh,
            tc.tile_pool(name="psa", bufs=1, space="PSUM") as psa,
            tc.tile_pool(name="pso", bufs=1, space="PSUM") as pso,
        ):
            nc.gpsimd.load_library(mlp_lib)
            w1_t = cpool.tile([D, 64], F16)
            nc.sync.dma_start(out=w1_t[:], in_=w1_d[:])
            b1_t = cpool.tile([64, 1], F32)
            nc.sync.dma_start(out=b1_t[:], in_=b1_d[:])
            w2_t = cpool.tile([64, 8], F16)
            nc.sync.dma_start(out=w2_t[:], in_=w2_d[:])
            b2_t = cpool.tile([8, 1], F32)
            nc.sync.dma_start(out=b2_t[:], in_=b2_d[:])
            mean_t = cpool.tile([8, 1], F16)
            nc.sync.dma_start(out=mean_t[:], in_=mean_d[:])
            iota_t = cpool.tile([P, SLOTS], F16)
            nc.sync.dma_start(out=iota_t[:], in_=iota_d[:])

            featsT = fpool.tile([P, nslots], F16)
            out_sb = opool.tile([1, nslots], F32)

            def emit_mlp(lo, hi):
                for j in range(lo, hi):
                    cols = slice(j * mlp_chunk, (j + 1) * mlp_chunk)
                    ph = psh.tile([64, mlp_chunk], F32, tag="ph")
                    nc.tensor.matmul(out=ph[:], lhsT=w1_t[:],
                                     rhs=featsT[:, cols], start=True, stop=True)
                    hr = tpool.tile([64, mlp_chunk], F16, tag="hr")
                    nc.scalar.activation(out=hr[:], in_=ph[:], func=AF.Relu,
                                         bias=b1_t[:])
                    pa = psa.tile([8, mlp_chunk], F32, tag="pa")
                    nc.tensor.matmul(out=pa[:], lhsT=w2_t[:], rhs=hr[:],
                                     start=True, stop=True)
                    sg = tpool.tile([8, mlp_chunk], F16, tag="sg")
                    nc.scalar.activation(out=sg[:], in_=pa[:], func=AF.Sigmoid,
                                         bias=b2_t[:])
                    nc.vector.tensor_scalar(out=sg[:], in0=sg[:],
                                            scalar1=float(SIG_LO),
                                            scalar2=float(SIG_HI),
                                            op0=OP.max, op1=OP.min)
                    po = pso.tile([1, mlp_chunk], F32, tag="po")
                    nc.tensor.matmul(out=po[:], lhsT=mean_t[:], rhs=sg[:],
                                     start=True, stop=True)
                    nc.scalar.activation(out=out_sb[:, cols], in_=po[:],
                                         func=AF.Copy, bias=0.1)

            for _r in range(repeat):
                bin0 = 0
                mlp_done = 0
                for kb_ch in chunk_plan:
                    # MLP for completed slots (previous chunks) goes first so
                    # PE isn't head-of-line blocked behind gather-dependent
                    # segment matmuls
                    avail = (bin0 * SLOTS) // mlp_chunk
                    emit_mlp(mlp_done, avail)
                    mlp_done = avail
                    nid = kb_ch * BINCAP
                    it = ipool.tile([P, N_SHARDS, KBMAX * IW], I16, tag="idx")
                    nc.sync.dma_start(
                        out=it[:, :, :kb_ch * IW],
                        in_=gidx[:, :, bin0 * IW:(bin0 + kb_ch) * IW])
                    sw = spool.tile([P, 2, KBMAX * N_SHARDS], F16, tag="sw")
                    nc.sync.dma_start(
                        out=sw[:, :, :kb_ch * N_SHARDS],
                        in_=slotw_d[:, :, bin0 * N_SHARDS:(bin0 + kb_ch) * N_SHARDS])
                    gts = []
                    for s in range(N_SHARDS):
                        G = gpool.tile([P, KBMAX, D], F8, tag="G")
                        _dma_gather_raw(
                            nc.gpsimd, G[:, :kb_ch, :], xs[s][:, 0:D],
                            it[:, s, :kb_ch * IW],
                            nid, D, XPAD, queue_num=s)
                        gts.append(G)
                    # build the scaled one-hot on-device:
                    #   m4[p, j, t] = (slot[p, j] == t) * w[p, j]
                    NBc = kb_ch * N_SHARDS
                    eq = epool.tile([P, KBMAX * N_SHARDS, SLOTS], F16, tag="eq")
                    slot_b = sw[:, 0, :NBc].unsqueeze(2).broadcast_to(
                        [P, NBc, SLOTS])
                    iota_b = iota_t[:].unsqueeze(1).broadcast_to([P, NBc, SLOTS])
                    nc.vector.tensor_tensor(out=eq[:, :NBc, :], in0=slot_b,
                                            in1=iota_b, op=OP.is_equal)
                    m4c = mpool.tile([P, KBMAX * N_SHARDS, SLOTS], F8, tag="m4")
                    w_b = sw[:, 1, :NBc].unsqueeze(2).broadcast_to(
                        [P, NBc, SLOTS])
                    nc.vector.tensor_tensor(out=m4c[:, :NBc, :],
                                            in0=eq[:, :NBc, :], in1=w_b,
                                            op=OP.mult)
                    GRP = 8  # bins per PSUM bank (8 x 64 f32 = one 2KB bank)
                    for k in range(kb_ch):
                        b = bin0 + k
                        if k % GRP == 0:
                            pf = psf.tile([P, GRP * SLOTS], F32, tag="pf")
                        col = (k % GRP) * SLOTS
                        for s in range(N_SHARDS):
                            nc.tensor.matmul(
                                out=pf[:, col:col + SLOTS],
                                lhsT=gts[s][:, k, :],
                                rhs=m4c[:, k * N_SHARDS + s, :],
                                start=(s == 0), stop=(s == N_SHARDS - 1))
                        if k % GRP == GRP - 1 or k == kb_ch - 1:
                            ncols = (k % GRP) + 1
                            lo_b = b - ncols + 1
                            nc.scalar.copy(
                                out=featsT[:, lo_b * SLOTS:(b + 1) * SLOTS],
                                in_=pf[:, :ncols * SLOTS])
                    bin0 += kb_ch
                emit_mlp(mlp_done, nslots // mlp_chunk)
            nc.sync.dma_start(out=out_d[:], in_=out_sb[:])
    nc.compile()
    return nc


# ---------------------------------------------------------------- entry point

def _host_fallback(out, segs, x, node_idx, hyperedge_idx, W1, b1, W2, b2):
    for s in segs:
        rows = x[node_idx[hyperedge_idx == s]]
        feats = rows.mean(axis=0) if len(rows) else np.zeros(IN_DIM, np.float32)
        h = np.maximum(np.einsum("d,hdk->hk", feats, W1) + b1, 0.0)
        alpha = np.einsum("hk,hk->h", h, W2) + b2
        w = 1.0 / (1.0 + np.exp(-np.clip(alpha, -5, 5)))
        out[s] = w.mean() * 0.9 + 0.1


def kernel(x, node_idx, hyperedge_idx, W1, b1, W2, b2):
    x = np.asarray(x, np.float32)
    node_idx = np.asarray(node_idx)
    hyperedge_idx = np.asarray(hyperedge_idx)
    W1 = np.asarray(W1, np.float32); b1 = np.asarray(b1, np.float32)
    W2 = np.asarray(W2, np.float32); b2 = np.asarray(b2, np.float32)

    packed = _pack(node_idx, hyperedge_idx)
    m = packed["meta"]
    consts = _make_mlp_consts(W1, b1, W2, b2)
    nc = build_nc(m["nbins"], N_CORES)
    in_maps = [_make_in_map(c, x, packed, consts) for c in range(N_CORES)]
    res = run_bass_kernel_spmd(nc, in_maps, list(range(N_CORES)))

    out = np.full(NUM_HYPEREDGES, np.nan, dtype=np.float32)
    om = packed["out_map"].reshape(N_CORES, -1)
    for c in range(N_CORES):
        core_out = res.results[c]["out"].reshape(-1)
        v = om[c] >= 0
        out[om[c][v]] = core_out[v]
    if len(packed["fallback"]):
        _host_fallback(out, packed["fallback"], x, node_idx, hyperedge_idx,
                       W1, b1, W2, b2)
    assert not np.isnan(out).any()
    return out
